# revision 15
# baseline (speedup 1.0000x reference)
"""AtomNet message-passing kernel for 8 Trainium2 NeuronCores.

Data-parallel over graphs: batch ids are sorted, KNN is batch-restricted, so
graph g goes to core g (8 graphs, 8 cores), no cross-core edges.

Per-core pipeline (all compute on device):
  1. transform: out = leaky(feat @ tw1 + tb1) @ tw2 + tb2        (PE + ACT/DVE)
  2. atom-atom KNN (k=16, self dropped) via PE distance matmul +
     DVE max8/max_index/match_replace rounds on negated d2
  3. 3 atom MP layers, decomposed edge MLP:
       msg_i = (sum_j leaky(C1_i + S1[idx_ij] + d_ij*r + b1)) @ w2 + 16*b2
       C1 = ctr @ w1[:128], S1 = src @ w1[128:256], r = w1[256]
     S1 table -> DRAM, per-edge rows gathered with indirect DMA (bf16),
     leaky+sum on DVE slabs, second matmul + groupnorm + residual.
  4. point-atom KNN, 3 point MP layers same way (src = final atom emb).
"""

import math
import os
from contextlib import ExitStack

import numpy as np

import concourse.bass as bass
import concourse.bacc as bacc_mod
import concourse.mybir as mybir
import concourse.tile as tile
from concourse.bass_utils import run_bass_kernel_spmd
from concourse.masks import make_identity
from concourse.tile import TileContext

P = 128
D = 128
K = 16
L = 3
H = 2 * D + 1  # 257
HP = 264  # padded gather row (bf16 elems)
SLOPE = 0.2
NEG_BIG = -3.0e38
F32 = mybir.dt.float32
BF16 = mybir.dt.bfloat16
U32 = mybir.dt.uint32
AX = mybir.AxisListType
ALU = mybir.AluOpType
ACTF = mybir.ActivationFunctionType


def _leaky_np(z):
    return np.where(z >= 0, z, SLOPE * z)


def _round_up(n, m):
    return ((n + m - 1) // m) * m


# ----------------------------------------------------------------------------
# device graph
# ----------------------------------------------------------------------------

def build_graph(NA_S, NX_S):
    NTA = NA_S // P
    NTX = NX_S // P
    nc = bacc_mod.Bacc()

    # ---- dram parameters -------------------------------------------------
    featT = nc.declare_dram_parameter("featT", [D, NA_S], F32, isOutput=False)
    a_q = nc.declare_dram_parameter("a_q", [4, NA_S], F32, isOutput=False)
    a_db = nc.declare_dram_parameter("a_db", [4, NA_S], F32, isOutput=False)
    a_qn = nc.declare_dram_parameter("a_qn", [NA_S, 1], F32, isOutput=False)
    x_q = nc.declare_dram_parameter("x_q", [4, NX_S], F32, isOutput=False)
    x_qn = nc.declare_dram_parameter("x_qn", [NX_S, 1], F32, isOutput=False)
    tw1 = nc.declare_dram_parameter("tw1", [D, D], F32, isOutput=False)
    tw2 = nc.declare_dram_parameter("tw2", [D, D], F32, isOutput=False)
    tb1 = nc.declare_dram_parameter("tb1", [D, 1], F32, isOutput=False)
    tb2 = nc.declare_dram_parameter("tb2", [D, 1], F32, isOutput=False)
    lw = {}
    for sg in ("aa", "ae"):
        for l in range(L):
            for nm, shp in (
                ("c1w", [D, H]), ("s1w", [D, H]), ("rrep", [P, H]),
                ("b1rep", [P, H]), ("w2a", [D, D]), ("w2b", [D, D]),
                ("w2c", [1, D]), ("b2rep", [P, D]), ("gwrep", [P, D]),
                ("gbrep", [P, D]),
            ):
                key = f"{sg}_{nm}_{l}"
                lw[key] = nc.declare_dram_parameter(key, shp, F32, isOutput=False)
    identbf_d = nc.declare_dram_parameter("identbf", [P, P], BF16, isOutput=False)
    identf_d = nc.declare_dram_parameter("identf32", [P, P], F32, isOutput=False)
    epsc_d = nc.declare_dram_parameter("epsc", [P, 1], F32, isOutput=False)
    out_ext = nc.declare_dram_parameter("out", [NX_S, D], F32, isOutput=True)

    table_dram = [nc.dram_tensor(f"table{i}", [NA_S, HP], BF16) for i in range(2)]

    ctx = ExitStack()
    tc = ctx.enter_context(TileContext(nc))
    const_p = ctx.enter_context(tc.tile_pool(name="const", bufs=1))
    wpool = ctx.enter_context(tc.tile_pool(name="weights", bufs=1))
    emb_p = ctx.enter_context(tc.tile_pool(name="emb", bufs=1))
    big_p = ctx.enter_context(tc.tile_pool(name="big", bufs=2))
    slab_p = ctx.enter_context(tc.tile_pool(name="slab", bufs=2))
    small_p = ctx.enter_context(tc.tile_pool(name="small", bufs=3))
    knn_p = ctx.enter_context(tc.tile_pool(name="knn", bufs=1))
    psum_p = ctx.enter_context(tc.tile_pool(name="psum", bufs=2, space="PSUM"))
    psum2_p = ctx.enter_context(tc.tile_pool(name="psum2", bufs=2, space="PSUM"))
    psumc_p = ctx.enter_context(tc.tile_pool(name="psumc", bufs=2, space="PSUM"))
    psumm_p = ctx.enter_context(tc.tile_pool(name="psumm", bufs=2, space="PSUM"))

    def dma(dst, src):
        nc.sync.dma_start(out=dst, in_=src)

    # ---- constants / weights to SBUF ------------------------------------
    ident = const_p.tile([P, P], BF16, tag="identf")
    dma(ident[:], identbf_d[:])
    identf32 = const_p.tile([P, P], F32, tag="identf32")
    dma(identf32[:], identf_d[:])
    eps_col = const_p.tile([P, 1], F32, tag="eps")
    dma(eps_col[:], epsc_d[:])

    sb = {}
    for key, hnd in lw.items():
        t = wpool.tile(list(hnd.shape), F32, tag=key)
        dma(t[:], hnd[:])
        sb[key] = t
    for nm, hnd in (("tw1", tw1), ("tw2", tw2), ("tb1", tb1), ("tb2", tb2),
                    ("featT", featT), ("a_q", a_q), ("a_db", a_db),
                    ("x_q", x_q)):
        t = wpool.tile(list(hnd.shape), F32, tag=nm)
        dma(t[:], hnd[:])
        sb[nm] = t
    # qn columns wrapped to [P, NT]
    a_qn_t = wpool.tile([P, NTA], F32, tag="a_qn")
    dma(a_qn_t[:], a_qn[:].rearrange("(t p) o -> p (t o)", p=P))
    x_qn_t = wpool.tile([P, NTX], F32, tag="x_qn")
    dma(x_qn_t[:], x_qn[:].rearrange("(t p) o -> p (t o)", p=P))

    rrep = {(sg, l): sb[f"{sg}_rrep_{l}"] for sg in ("aa", "ae") for l in range(L)}
    gwrep = {(sg, l): sb[f"{sg}_gwrep_{l}"] for sg in ("aa", "ae") for l in range(L)}
    gbrep = {(sg, l): sb[f"{sg}_gbrep_{l}"] for sg in ("aa", "ae") for l in range(L)}

    def leaky_inplace(t, ap, tmp_pool=small_p):
        """t[ap...] = leaky(t) via mul+max. ap is an AP on an SBUF tile."""
        tmp = tmp_pool.tile([ap.shape[0], ap.free_size()], ap.dtype)
        nc.vector.tensor_scalar_mul(tmp[:], ap, SLOPE)
        nc.vector.tensor_tensor(out=ap, in0=ap, in1=tmp[:], op=ALU.max)

    # ---- stage 1: transform atoms ---------------------------------------
    # a_embT [D, NA_S] f32, feature-on-partition
    a_embT = emb_p.tile([D, NA_S], F32, tag="a_embT")
    CH = 512
    for c0 in range(0, NA_S, CH):
        cw = min(CH, NA_S - c0)
        ps = psum_p.tile([P, cw], F32, tag="mm")
        nc.tensor.matmul(ps[:], lhsT=sb["tw1"][:], rhs=sb["featT"][:, c0:c0 + cw],
                         start=True, stop=True)
        h = big_p.tile([P, cw], F32, tag="h_trans")
        nc.scalar.activation(h[:], ps[:], ACTF.Identity, bias=sb["tb1"][:, 0:1])
        leaky_inplace(h, h[:], big_p)
        ps2 = psum_p.tile([P, cw], F32, tag="mm")
        nc.tensor.matmul(ps2[:], lhsT=sb["tw2"][:], rhs=h[:], start=True, stop=True)
        nc.scalar.activation(a_embT[:, c0:c0 + cw], ps2[:], ACTF.Identity,
                             bias=sb["tb2"][:, 0:1])

    # atom emb tiles [ctr, D] f32 (transpose of a_embT chunks)
    a_emb = []
    for ct in range(NTA):
        pst = psum_p.tile([P, P], F32, tag="mm")
        nc.tensor.transpose(pst[:], in_=a_embT[:, ct * P:(ct + 1) * P],
                            identity=identf32[:])
        t = emb_p.tile([P, D], F32, tag=f"a_emb{ct}")
        nc.vector.tensor_copy(t[:], pst[:])
        a_emb.append(t)

    # ---- KNN -------------------------------------------------------------
    def knn(q_aug, qn_t, NT, diag_kill):
        """returns (idx tiles [P,16] u32, dist tiles [P,16] f32) per ctr tile."""
        idx_tiles, dst_tiles = [], []
        for ct in range(NT):
            nd2 = big_p.tile([P, NA_S], F32, tag="nd2")
            for c0 in range(0, NA_S, CH):
                cw = min(CH, NA_S - c0)
                ps = psum2_p.tile([P, cw], F32, tag="t")
                nc.tensor.matmul(ps[:], lhsT=q_aug[:, ct * P:(ct + 1) * P],
                                 rhs=sb["a_db"][:, c0:c0 + cw], start=True, stop=True)
                nc.scalar.activation(nd2[:, c0:c0 + cw], ps[:], ACTF.Identity,
                                     bias=qn_t[:, ct:ct + 1])
            if diag_kill:
                nc.gpsimd.affine_select(
                    out=nd2[:], in_=nd2[:], compare_op=ALU.not_equal,
                    fill=NEG_BIG, base=-P * ct,
                    pattern=[[1, NA_S]], channel_multiplier=-1)
            idx = knn_p.tile([P, K], U32, tag=f"idx_{diag_kill}_{ct}")
            dst = knn_p.tile([P, K], F32, tag=f"dst_{diag_kill}_{ct}")
            m8 = small_p.tile([P, 8], F32, tag="m8")
            nc.vector.max(out=m8[:], in_=nd2[:])
            nc.vector.max_index(idx[:, 0:8], m8[:], nd2[:])
            nc.vector.tensor_scalar_mul(dst[:, 0:8], m8[:], -1.0)
            nc.vector.match_replace(out=nd2[:], in_to_replace=m8[:],
                                    in_values=nd2[:], imm_value=NEG_BIG)
            m8b = small_p.tile([P, 8], F32, tag="m8b")
            nc.vector.max(out=m8b[:], in_=nd2[:])
            nc.vector.max_index(idx[:, 8:16], m8b[:], nd2[:])
            nc.vector.tensor_scalar_mul(dst[:, 8:16], m8b[:], -1.0)
            idx_tiles.append(idx)
            dst_tiles.append(dst)
        return idx_tiles, dst_tiles

    a_idx, a_dst = knn(sb["a_q"], a_qn_t, NTA, diag_kill=True)
    x_idx, x_dst = knn(sb["x_q"], x_qn_t, NTX, diag_kill=False)

    # ---- MP layer --------------------------------------------------------
    def mp_layer(sg, l, srcT, ctrT, ctr_tiles, idx_tiles, dst_tiles, NT, tdram,
                 out_is_final=False):
        """One message-passing layer.
        srcT  : [D, NA_S] f32 source features (feature-on-partition)
        ctrT  : [D, NT*P] f32 center features (feature-on-partition)
        ctr_tiles: list of [P, D] f32 center tiles (residual)
        returns (new ctrT tile, new ctr tiles) unless out_is_final: writes out.
        """
        # source table -> DRAM (bf16)
        tbl = big_p.tile([P, NTA, HP], BF16, tag="tbl")
        nc.vector.memset(tbl[:, :, H:], 0.0)
        for st in range(NTA):
            ps = psum_p.tile([P, H], F32, tag="mm")
            nc.tensor.matmul(ps[:], lhsT=srcT[:, st * P:(st + 1) * P],
                             rhs=sb[f"{sg}_s1w_{l}"][:], start=True, stop=True)
            nc.vector.tensor_tensor(out=tbl[:, st, 0:H], in0=ps[:],
                                    in1=sb[f"{sg}_b1rep_{l}"][:], op=ALU.add)
        dma(tdram[:].rearrange("(t p) f -> p t f", p=P), tbl[:])

        new_T = None if out_is_final else emb_p.tile(
            [D, NT * P], F32, tag=f"{sg}_embT_{l % 2}")
        new_tiles = []
        for ct in range(NT):
            # C1 + b1 (bf16 [P, H])
            ps = psumc_p.tile([P, H], F32, tag="c1")
            nc.tensor.matmul(ps[:], lhsT=ctrT[:, ct * P:(ct + 1) * P],
                             rhs=sb[f"{sg}_c1w_{l}"][:], start=True, stop=True)
            c1 = small_p.tile([P, H], BF16, tag="c1")
            nc.scalar.copy(c1[:], ps[:])

            # gather slab
            g = slab_p.tile([P, K, HP], BF16, tag="g")
            for j in range(K):
                nc.gpsimd.indirect_dma_start(
                    out=g[:, j, :], out_offset=None, in_=tdram[:],
                    in_offset=bass.IndirectOffsetOnAxis(
                        ap=idx_tiles[ct][:, j:j + 1], axis=0))

            # base slab: c1 replicated (doubling) then += d x r
            base = slab_p.tile([P, K, HP], BF16, tag="base")
            nc.vector.tensor_copy(base[:, 0, 0:H], c1[:])
            nc.vector.memset(base[:, 0, H:], 0.0)
            for step in (1, 2, 4, 8):
                nc.vector.tensor_copy(base[:, step:2 * step, :], base[:, 0:step, :])
            dr = slab_p.tile([P, K, HP], BF16, tag="dr")
            for j in range(K):
                nc.vector.tensor_scalar_mul(dr[:, j, 0:H], rrep[(sg, l)][:],
                                            dst_tiles[ct][:, j:j + 1])
            nc.vector.tensor_tensor(out=base[:, :, 0:H],
                                    in0=base[:, :, 0:H], in1=dr[:, :, 0:H],
                                    op=ALU.add)
            # z = g + base ; leaky ; sum over j
            nc.vector.tensor_tensor(out=g[:], in0=g[:], in1=base[:], op=ALU.add)
            lk = slab_p.tile([P, K, HP], BF16, tag="lk")
            nc.vector.tensor_scalar_mul(lk[:], g[:], SLOPE)
            nc.vector.tensor_tensor(out=g[:], in0=g[:], in1=lk[:], op=ALU.max)
            for step in (8, 4, 2, 1):
                nc.vector.tensor_tensor(out=g[:, 0:step, :], in0=g[:, 0:step, :],
                                        in1=g[:, step:2 * step, :], op=ALU.add)
            # u = g[:, 0, 0:H] bf16 -> transpose chunks
            u0 = psum2_p.tile([P, P], BF16, tag="t")
            nc.tensor.transpose(u0[:], in_=g[:, 0, 0:P], identity=ident[:])
            u1 = psum2_p.tile([P, P], BF16, tag="t")
            nc.tensor.transpose(u1[:], in_=g[:, 0, P:2 * P], identity=ident[:])
            u2 = psum2_p.tile([1, P], BF16, tag="t")
            nc.tensor.transpose(u2[:], in_=g[:, 0, 2 * P:H], identity=ident[:])
            u0s = small_p.tile([P, P], BF16, tag="u0s")
            nc.vector.tensor_copy(u0s[:], u0[:])
            u1s = small_p.tile([P, P], BF16, tag="u1s")
            nc.vector.tensor_copy(u1s[:], u1[:])
            u2s = small_p.tile([1, P], BF16, tag="u2s")
            nc.vector.tensor_copy(u2s[:], u2[:])

            w2a_b = small_p.tile([P, D], BF16, tag="w2ab")
            nc.vector.tensor_copy(w2a_b[:], sb[f"{sg}_w2a_{l}"][:])
            w2b_b = small_p.tile([P, D], BF16, tag="w2bb")
            nc.vector.tensor_copy(w2b_b[:], sb[f"{sg}_w2b_{l}"][:])
            w2c_b = small_p.tile([1, D], BF16, tag="w2cb")
            nc.vector.tensor_copy(w2c_b[:], sb[f"{sg}_w2c_{l}"][:])

            msg = psumm_p.tile([P, D], F32, tag="msg")
            nc.tensor.matmul(msg[:], lhsT=u0s[:], rhs=w2a_b[:], start=True, stop=False)
            nc.tensor.matmul(msg[:], lhsT=u1s[:], rhs=w2b_b[:], start=False, stop=False)
            nc.tensor.matmul(msg[:], lhsT=u2s[:], rhs=w2c_b[:], start=False, stop=True)

            # groupnorm (2 groups of 64) + leaky + residual
            y = small_p.tile([P, D], F32, tag="y")
            nc.vector.tensor_tensor(out=y[:], in0=msg[:],
                                    in1=sb[f"{sg}_b2rep_{l}"][:], op=ALU.add)
            var = small_p.tile([P, 2], F32, tag="var")
            for h in range(2):
                hs = slice(h * 64, (h + 1) * 64)
                mu = small_p.tile([P, 1], F32, tag="mu")
                nc.vector.tensor_reduce(mu[:], y[:, hs], axis=AX.X, op=ALU.add)
                nc.vector.tensor_scalar_mul(mu[:], mu[:], 1.0 / 64.0)
                nc.vector.tensor_scalar(y[:, hs], y[:, hs], mu[:, 0:1], None,
                                        op0=ALU.subtract)
                sq = small_p.tile([P, 64], F32, tag="sq")
                nc.vector.tensor_tensor(out=sq[:], in0=y[:, hs], in1=y[:, hs],
                                        op=ALU.mult)
                nc.vector.tensor_reduce(var[:, h:h + 1], sq[:], axis=AX.X, op=ALU.add)
            rstd = small_p.tile([P, 2], F32, tag="rstd")
            nc.scalar.activation(rstd[:], var[:], ACTF.Sqrt, bias=eps_col[:, 0:1],
                                 scale=1.0 / 64.0)
            nc.vector.reciprocal(rstd[:], rstd[:])
            for h in range(2):
                hs = slice(h * 64, (h + 1) * 64)
                nc.vector.tensor_scalar(y[:, hs], y[:, hs], rstd[:, h:h + 1], None,
                                        op0=ALU.mult)
            nc.vector.tensor_tensor(out=y[:], in0=y[:], in1=gwrep[(sg, l)][:],
                                    op=ALU.mult)
            nc.vector.tensor_tensor(out=y[:], in0=y[:], in1=gbrep[(sg, l)][:],
                                    op=ALU.add)
            leaky_inplace(y, y[:])
            nc.vector.tensor_tensor(out=y[:], in0=y[:], in1=ctr_tiles[ct][:],
                                    op=ALU.add)
            if out_is_final:
                dma(out_ext[ct * P:(ct + 1) * P, :], y[:])
                new_tiles.append(None)
            else:
                nt = emb_p.tile([P, D], F32, tag=f"{sg}_emb{ct}_{l % 2}")
                nc.vector.tensor_copy(nt[:], y[:])
                new_tiles.append(nt)
                pst = psum2_p.tile([P, P], F32, tag="t")
                nc.tensor.transpose(pst[:], in_=nt[:], identity=identf32[:])
                nc.vector.tensor_copy(new_T[:, ct * P:(ct + 1) * P], pst[:])
        return new_T, new_tiles

    # ---- atom stage ------------------------------------------------------
    curT, cur = a_embT, a_emb
    for l in range(L):
        curT, cur = mp_layer("aa", l, curT, curT, cur, a_idx, a_dst, NTA,
                             table_dram[l % 2])
    a_finT = curT

    # ---- point stage ------------------------------------------------------
    p_embT = emb_p.tile([D, NX_S], F32, tag="p_embT")
    nc.vector.memset(p_embT[:], 1.0)
    p_emb = []
    for ct in range(NTX):
        t = emb_p.tile([P, D], F32, tag=f"p_emb{ct}")
        nc.vector.memset(t[:], 1.0)
        p_emb.append(t)
    curT, cur = p_embT, p_emb
    for l in range(L):
        curT, cur = mp_layer("ae", l, a_finT, curT, cur, x_idx, x_dst, NTX,
                             table_dram[l % 2], out_is_final=(l == L - 1))

    ctx.close()
    nc.finalize()
    return nc


# ----------------------------------------------------------------------------
# host wrapper
# ----------------------------------------------------------------------------

def kernel(**inputs):
    xyz = np.asarray(inputs["xyz"], np.float32)
    atom_xyz = np.asarray(inputs["atom_xyz"], np.float32)
    atom_features = np.asarray(inputs["atom_features"], np.float32)
    batch = np.asarray(inputs["batch"]).astype(np.int64)
    atom_batch = np.asarray(inputs["atom_batch"]).astype(np.int64)

    NX, NA, B = xyz.shape[0], atom_xyz.shape[0], 8
    cx = np.bincount(batch, minlength=B)
    ca = np.bincount(atom_batch, minlength=B)
    ox = np.concatenate([[0], np.cumsum(cx)])
    oa = np.concatenate([[0], np.cumsum(ca)])
    NA_S = max(_round_up(int(ca.max()), P), P)
    NX_S = max(_round_up(int(cx.max()), P), P)

    nc = build_graph(NA_S, NX_S)

    shared = {
        "tw1": np.ascontiguousarray(np.asarray(inputs["tw1"], np.float32)),
        "tw2": np.ascontiguousarray(np.asarray(inputs["tw2"], np.float32)),
        "tb1": np.asarray(inputs["tb1"], np.float32)[:, None].copy(),
        "tb2": np.asarray(inputs["tb2"], np.float32)[:, None].copy(),
    }
    shared.update(lay_dict(inputs, "aa"))
    shared.update(lay_dict(inputs, "ae"))
    import ml_dtypes
    shared["identbf"] = np.eye(P, dtype=ml_dtypes.bfloat16)
    shared["identf32"] = np.eye(P, dtype=np.float32)
    shared["epsc"] = np.full((P, 1), 1e-5, np.float32)

    in_maps = []
    for g in range(B):
        na, nx = int(ca[g]), int(cx[g])
        ax = atom_xyz[oa[g]:oa[g] + na]
        px = xyz[ox[g]:ox[g] + nx]
        af = atom_features[oa[g]:oa[g] + na]

        featT = np.zeros((D, NA_S), np.float32)
        featT[:, :na] = af.T
        a_q = np.zeros((4, NA_S), np.float32)
        a_q[3, :] = 1.0
        a_q[:3, :na] = 2.0 * ax.T
        a_db = np.zeros((4, NA_S), np.float32)
        a_db[3, :] = -1e30
        a_db[:3, :na] = ax.T
        a_db[3, :na] = -np.sum(ax * ax, axis=1)
        a_qn = np.zeros((NA_S, 1), np.float32)
        a_qn[:na, 0] = -np.sum(ax * ax, axis=1)
        x_q = np.zeros((4, NX_S), np.float32)
        x_q[3, :] = 1.0
        x_q[:3, :nx] = 2.0 * px.T
        x_qn = np.zeros((NX_S, 1), np.float32)
        x_qn[:nx, 0] = -np.sum(px * px, axis=1)

        m = {"featT": featT, "a_q": a_q, "a_db": a_db, "a_qn": a_qn,
             "x_q": x_q, "x_qn": x_qn}
        m.update(shared)
        in_maps.append(m)

    trace = bool(os.environ.get("ATOM_TRACE"))
    rr = run_bass_kernel_spmd(nc, in_maps, core_ids=list(range(B)), trace=trace)
    if trace and rr.exec_time_ns:
        print(f"HW exec time: {rr.exec_time_ns} ns", flush=True)
    if os.environ.get("ATOM_TIME"):
        import time as _t
        ts = []
        for _ in range(int(os.environ.get("ATOM_TIME", "3"))):
            t0 = _t.perf_counter()
            rr = run_bass_kernel_spmd(nc, in_maps, core_ids=list(range(B)))
            ts.append(_t.perf_counter() - t0)
        print(f"wall exec times: {[f'{t*1e3:.1f}ms' for t in ts]}", flush=True)
        print(f"HW exec time: {min(ts)*1e9:.0f} ns", flush=True)
    res = rr.results
    out = np.empty((NX, D), np.float32)
    for g in range(B):
        out[ox[g]:ox[g] + int(cx[g])] = res[g]["out"][:int(cx[g])]
    return out


def lay_dict(inputs, sg):
    w1 = np.asarray(inputs[f"{sg}_w1"], np.float32)
    b1 = np.asarray(inputs[f"{sg}_b1"], np.float32)
    w2 = np.asarray(inputs[f"{sg}_w2"], np.float32)
    b2 = np.asarray(inputs[f"{sg}_b2"], np.float32)
    gw = np.asarray(inputs[f"{sg}_gw"], np.float32)
    gb = np.asarray(inputs[f"{sg}_gb"], np.float32)
    d = {}
    for l in range(L):
        d[f"{sg}_c1w_{l}"] = np.ascontiguousarray(w1[l][:D, :])
        d[f"{sg}_s1w_{l}"] = np.ascontiguousarray(w1[l][D:2 * D, :])
        d[f"{sg}_rrep_{l}"] = np.broadcast_to(w1[l][2 * D], (P, H)).copy()
        d[f"{sg}_b1rep_{l}"] = np.broadcast_to(b1[l], (P, H)).copy()
        d[f"{sg}_w2a_{l}"] = np.ascontiguousarray(w2[l][:D, :])
        d[f"{sg}_w2b_{l}"] = np.ascontiguousarray(w2[l][D:2 * D, :])
        d[f"{sg}_w2c_{l}"] = w2[l][2 * D:2 * D + 1, :].copy()
        d[f"{sg}_b2rep_{l}"] = np.broadcast_to(K * b2[l], (P, D)).copy()
        d[f"{sg}_gwrep_{l}"] = np.broadcast_to(gw[l], (P, D)).copy()
        d[f"{sg}_gbrep_{l}"] = np.broadcast_to(gb[l], (P, D)).copy()
    return d


# revision 16
# speedup vs baseline: 1.1178x; 1.1178x over previous
"""AtomNet message-passing kernel for 8 Trainium2 NeuronCores.

Data-parallel over graphs: batch ids are sorted, KNN is batch-restricted, so
graph g goes to core g (8 graphs, 8 cores), no cross-core edges.

Per-core pipeline (all compute on device):
  1. transform: out = leaky(feat @ tw1 + tb1) @ tw2 + tb2        (PE + ACT/DVE)
  2. atom-atom KNN (k=16, self dropped) via PE distance matmul +
     DVE max8/max_index/match_replace rounds on negated d2
  3. 3 atom MP layers, decomposed edge MLP:
       msg_i = (sum_j leaky(C1_i + S1[idx_ij] + d_ij*r + b1)) @ w2 + 16*b2
       C1 = ctr @ w1[:128], S1 = src @ w1[128:256], r = w1[256]
     S1 table -> DRAM, per-edge rows gathered with indirect DMA (bf16),
     leaky+sum on DVE slabs, second matmul + groupnorm + residual.
  4. point-atom KNN, 3 point MP layers same way (src = final atom emb).
"""

import math
import os
from contextlib import ExitStack

import ml_dtypes
import numpy as np

import concourse.bass as bass
import concourse.bacc as bacc_mod
import concourse.mybir as mybir
import concourse.tile as tile
from concourse.bass_utils import run_bass_kernel_spmd
from concourse.masks import make_identity
from concourse.tile import TileContext

P = 128
D = 128
K = 16
L = 3
H = 2 * D + 1  # 257
HP = 264  # padded gather row (bf16 elems)
SLOPE = 0.2
NEG_BIG = -3.0e38
F32 = mybir.dt.float32
BF16 = mybir.dt.bfloat16
U32 = mybir.dt.uint32
AX = mybir.AxisListType
ALU = mybir.AluOpType
ACTF = mybir.ActivationFunctionType


def _leaky_np(z):
    return np.where(z >= 0, z, SLOPE * z)


def _round_up(n, m):
    return ((n + m - 1) // m) * m


# ----------------------------------------------------------------------------
# device graph
# ----------------------------------------------------------------------------

def build_graph(NA_S, NX_S):
    NTA = NA_S // P
    NTX = NX_S // P
    nc = bacc_mod.Bacc()

    # ---- dram parameters -------------------------------------------------
    featT = nc.declare_dram_parameter("featT", [D, NA_S], F32, isOutput=False)
    a_q = nc.declare_dram_parameter("a_q", [4, NA_S], F32, isOutput=False)
    a_db = nc.declare_dram_parameter("a_db", [4, NA_S], F32, isOutput=False)
    a_qn = nc.declare_dram_parameter("a_qn", [NA_S, 1], F32, isOutput=False)
    x_q = nc.declare_dram_parameter("x_q", [4, NX_S], F32, isOutput=False)
    x_qn = nc.declare_dram_parameter("x_qn", [NX_S, 1], F32, isOutput=False)
    tw1 = nc.declare_dram_parameter("tw1", [D, D], F32, isOutput=False)
    tw2 = nc.declare_dram_parameter("tw2", [D, D], F32, isOutput=False)
    tb1 = nc.declare_dram_parameter("tb1", [D, 1], F32, isOutput=False)
    tb2 = nc.declare_dram_parameter("tb2", [D, 1], F32, isOutput=False)
    lw = {}
    for sg in ("aa", "ae"):
        for l in range(L):
            for nm, shp in (
                ("c1w", [D, H]), ("s1w", [D, H]), ("rrep", [P, HP]),
                ("b1rep", [P, H]), ("w2a", [D, D]), ("w2b", [D, D]),
                ("w2c", [1, D]), ("b2rep", [P, D]), ("gwrep", [P, D]),
                ("gbrep", [P, D]),
            ):
                key = f"{sg}_{nm}_{l}"
                dt = BF16 if nm in ("rrep", "w2a", "w2b", "w2c") else F32
                lw[key] = nc.declare_dram_parameter(key, shp, dt, isOutput=False)
    identbf_d = nc.declare_dram_parameter("identbf", [P, P], BF16, isOutput=False)
    identf_d = nc.declare_dram_parameter("identf32", [P, P], F32, isOutput=False)
    epsc_d = nc.declare_dram_parameter("epsc", [P, 1], F32, isOutput=False)
    out_ext = nc.declare_dram_parameter("out", [NX_S, D], F32, isOutput=True)

    table_dram = [nc.dram_tensor(f"table{i}", [NA_S, HP], BF16) for i in range(2)]

    ctx = ExitStack()
    tc = ctx.enter_context(TileContext(nc))
    const_p = ctx.enter_context(tc.tile_pool(name="const", bufs=1))
    wpool = ctx.enter_context(tc.tile_pool(name="weights", bufs=1))
    emb_p = ctx.enter_context(tc.tile_pool(name="emb", bufs=1))
    big_p = ctx.enter_context(tc.tile_pool(name="big", bufs=2))
    slab_p = ctx.enter_context(tc.tile_pool(name="slab", bufs=2))
    small_p = ctx.enter_context(tc.tile_pool(name="small", bufs=3))
    knn_p = ctx.enter_context(tc.tile_pool(name="knn", bufs=1))
    psum_p = ctx.enter_context(tc.tile_pool(name="psum", bufs=2, space="PSUM"))
    psum2_p = ctx.enter_context(tc.tile_pool(name="psum2", bufs=2, space="PSUM"))
    psumc_p = ctx.enter_context(tc.tile_pool(name="psumc", bufs=2, space="PSUM"))
    psumm_p = ctx.enter_context(tc.tile_pool(name="psumm", bufs=2, space="PSUM"))

    def dma(dst, src):
        nc.sync.dma_start(out=dst, in_=src)

    # ---- constants / weights to SBUF ------------------------------------
    ident = const_p.tile([P, P], BF16, tag="identf")
    dma(ident[:], identbf_d[:])
    identf32 = const_p.tile([P, P], F32, tag="identf32")
    dma(identf32[:], identf_d[:])
    eps_col = const_p.tile([P, 1], F32, tag="eps")
    dma(eps_col[:], epsc_d[:])

    sb = {}
    for key, hnd in lw.items():
        t = wpool.tile(list(hnd.shape), hnd.dtype, tag=key)
        dma(t[:], hnd[:])
        sb[key] = t
    for nm, hnd in (("tw1", tw1), ("tw2", tw2), ("tb1", tb1), ("tb2", tb2),
                    ("featT", featT), ("a_q", a_q), ("a_db", a_db),
                    ("x_q", x_q)):
        t = wpool.tile(list(hnd.shape), F32, tag=nm)
        dma(t[:], hnd[:])
        sb[nm] = t
    # qn columns wrapped to [P, NT]
    a_qn_t = wpool.tile([P, NTA], F32, tag="a_qn")
    dma(a_qn_t[:], a_qn[:].rearrange("(t p) o -> p (t o)", p=P))
    x_qn_t = wpool.tile([P, NTX], F32, tag="x_qn")
    dma(x_qn_t[:], x_qn[:].rearrange("(t p) o -> p (t o)", p=P))

    rrep = {(sg, l): sb[f"{sg}_rrep_{l}"] for sg in ("aa", "ae") for l in range(L)}
    gwrep = {(sg, l): sb[f"{sg}_gwrep_{l}"] for sg in ("aa", "ae") for l in range(L)}
    gbrep = {(sg, l): sb[f"{sg}_gbrep_{l}"] for sg in ("aa", "ae") for l in range(L)}

    def leaky_inplace(t, ap, tmp_pool=small_p):
        """t[ap...] = leaky(t) via mul+max. ap is an AP on an SBUF tile."""
        tmp = tmp_pool.tile([ap.shape[0], ap.free_size()], ap.dtype)
        nc.vector.tensor_scalar_mul(tmp[:], ap, SLOPE)
        nc.vector.tensor_tensor(out=ap, in0=ap, in1=tmp[:], op=ALU.max)

    # ---- stage 1: transform atoms ---------------------------------------
    # a_embT [D, NA_S] f32, feature-on-partition
    a_embT = emb_p.tile([D, NA_S], F32, tag="a_embT")
    CH = 512
    for c0 in range(0, NA_S, CH):
        cw = min(CH, NA_S - c0)
        ps = psum_p.tile([P, cw], F32, tag="mm")
        nc.tensor.matmul(ps[:], lhsT=sb["tw1"][:], rhs=sb["featT"][:, c0:c0 + cw],
                         start=True, stop=True)
        h = big_p.tile([P, cw], F32, tag="h_trans")
        nc.scalar.activation(h[:], ps[:], ACTF.Identity, bias=sb["tb1"][:, 0:1])
        leaky_inplace(h, h[:], big_p)
        ps2 = psum_p.tile([P, cw], F32, tag="mm")
        nc.tensor.matmul(ps2[:], lhsT=sb["tw2"][:], rhs=h[:], start=True, stop=True)
        nc.scalar.activation(a_embT[:, c0:c0 + cw], ps2[:], ACTF.Identity,
                             bias=sb["tb2"][:, 0:1])

    # atom emb tiles [ctr, D] f32 (transpose of a_embT chunks)
    a_emb = []
    for ct in range(NTA):
        pst = psum_p.tile([P, P], F32, tag="mm")
        nc.tensor.transpose(pst[:], in_=a_embT[:, ct * P:(ct + 1) * P],
                            identity=identf32[:])
        t = emb_p.tile([P, D], F32, tag=f"a_emb{ct}")
        nc.vector.tensor_copy(t[:], pst[:])
        a_emb.append(t)

    # ---- KNN -------------------------------------------------------------
    def knn(q_aug, qn_t, NT, diag_kill):
        """returns (idx tiles [P,16] u32, dist tiles [P,16] f32) per ctr tile."""
        idx_tiles, dst_tiles = [], []
        for ct in range(NT):
            nd2 = big_p.tile([P, NA_S], F32, tag="nd2")
            for c0 in range(0, NA_S, CH):
                cw = min(CH, NA_S - c0)
                ps = psum2_p.tile([P, cw], F32, tag="t")
                nc.tensor.matmul(ps[:], lhsT=q_aug[:, ct * P:(ct + 1) * P],
                                 rhs=sb["a_db"][:, c0:c0 + cw], start=True, stop=True)
                nc.scalar.activation(nd2[:, c0:c0 + cw], ps[:], ACTF.Identity,
                                     bias=qn_t[:, ct:ct + 1])
            if diag_kill:
                nc.gpsimd.affine_select(
                    out=nd2[:], in_=nd2[:], compare_op=ALU.not_equal,
                    fill=NEG_BIG, base=-P * ct,
                    pattern=[[1, NA_S]], channel_multiplier=-1)
            idx = knn_p.tile([P, K], U32, tag=f"idx_{diag_kill}_{ct}")
            dst = knn_p.tile([P, K], F32, tag=f"dst_{diag_kill}_{ct}")
            m8 = small_p.tile([P, 8], F32, tag="m8")
            nc.vector.max(out=m8[:], in_=nd2[:])
            nc.vector.max_index(idx[:, 0:8], m8[:], nd2[:])
            nc.vector.tensor_scalar_mul(dst[:, 0:8], m8[:], -1.0)
            nc.vector.match_replace(out=nd2[:], in_to_replace=m8[:],
                                    in_values=nd2[:], imm_value=NEG_BIG)
            m8b = small_p.tile([P, 8], F32, tag="m8b")
            nc.vector.max(out=m8b[:], in_=nd2[:])
            nc.vector.max_index(idx[:, 8:16], m8b[:], nd2[:])
            nc.vector.tensor_scalar_mul(dst[:, 8:16], m8b[:], -1.0)
            idx_tiles.append(idx)
            dst_tiles.append(dst)
        return idx_tiles, dst_tiles

    a_idx, a_dst = knn(sb["a_q"], a_qn_t, NTA, diag_kill=True)
    x_idx, x_dst = knn(sb["x_q"], x_qn_t, NTX, diag_kill=False)

    # ---- MP layer --------------------------------------------------------
    def mp_layer(sg, l, srcT, ctrT, ctr_tiles, idx_tiles, dst_tiles, NT, tdram,
                 out_is_final=False):
        """One message-passing layer.
        srcT  : [D, NA_S] f32 source features (feature-on-partition)
        ctrT  : [D, NT*P] f32 center features (feature-on-partition)
        ctr_tiles: list of [P, D] f32 center tiles (residual)
        returns (new ctrT tile, new ctr tiles) unless out_is_final: writes out.
        """
        # source table -> DRAM (bf16)
        tbl = big_p.tile([P, NTA, HP], BF16, tag="tbl")
        nc.vector.memset(tbl[:, :, H:], 0.0)
        for st in range(NTA):
            ps = psum_p.tile([P, H], F32, tag="mm")
            nc.tensor.matmul(ps[:], lhsT=srcT[:, st * P:(st + 1) * P],
                             rhs=sb[f"{sg}_s1w_{l}"][:], start=True, stop=True)
            nc.vector.tensor_tensor(out=tbl[:, st, 0:H], in0=ps[:],
                                    in1=sb[f"{sg}_b1rep_{l}"][:], op=ALU.add)
        dma(tdram[:].rearrange("(t p) f -> p t f", p=P), tbl[:])

        new_T = None if out_is_final else emb_p.tile(
            [D, NT * P], F32, tag=f"{sg}_embT_{l % 2}")
        new_tiles = []
        for ct in range(NT):
            # C1 + b1 (bf16 [P, H])
            ps = psumc_p.tile([P, H], F32, tag="c1")
            nc.tensor.matmul(ps[:], lhsT=ctrT[:, ct * P:(ct + 1) * P],
                             rhs=sb[f"{sg}_c1w_{l}"][:], start=True, stop=True)
            c1 = small_p.tile([P, H], BF16, tag="c1")
            nc.scalar.copy(c1[:], ps[:])

            # base slab: g[:, j, :] = d_j * r (rrep padded, zeros past H), += c1
            g = slab_p.tile([P, K, HP], BF16, tag="g")
            for j in range(K):
                nc.vector.tensor_scalar_mul(g[:, j, :], rrep[(sg, l)][:],
                                            dst_tiles[ct][:, j:j + 1])
            for j in range(K):
                nc.vector.tensor_tensor(out=g[:, j, 0:H], in0=g[:, j, 0:H],
                                        in1=c1[:], op=ALU.add)
            # gather-accumulate source rows on top: z = base + s1[idx]
            for j in range(K):
                nc.gpsimd.indirect_dma_start(
                    out=g[:, j, :], out_offset=None, in_=tdram[:],
                    in_offset=bass.IndirectOffsetOnAxis(
                        ap=idx_tiles[ct][:, j:j + 1], axis=0),
                    compute_op=ALU.add)
            # leaky: max(z, 0.2 z); 0.2*z computed on ACT
            lk = slab_p.tile([P, K, HP], BF16, tag="lk")
            nc.scalar.activation(lk[:], g[:], ACTF.Identity, scale=0.2)
            nc.vector.tensor_tensor(out=g[:], in0=g[:], in1=lk[:], op=ALU.max)
            for step in (8, 4, 2, 1):
                nc.vector.tensor_tensor(out=g[:, 0:step, :], in0=g[:, 0:step, :],
                                        in1=g[:, step:2 * step, :], op=ALU.add)
            # u = g[:, 0, 0:H] bf16 -> transpose chunks
            u0 = psum2_p.tile([P, P], BF16, tag="t")
            nc.tensor.transpose(u0[:], in_=g[:, 0, 0:P], identity=ident[:])
            u1 = psum2_p.tile([P, P], BF16, tag="t")
            nc.tensor.transpose(u1[:], in_=g[:, 0, P:2 * P], identity=ident[:])
            u2 = psum2_p.tile([1, P], BF16, tag="t")
            nc.tensor.transpose(u2[:], in_=g[:, 0, 2 * P:H], identity=ident[:])
            u0s = small_p.tile([P, P], BF16, tag="u0s")
            nc.vector.tensor_copy(u0s[:], u0[:])
            u1s = small_p.tile([P, P], BF16, tag="u1s")
            nc.vector.tensor_copy(u1s[:], u1[:])
            u2s = small_p.tile([1, P], BF16, tag="u2s")
            nc.vector.tensor_copy(u2s[:], u2[:])

            msg = psumm_p.tile([P, D], F32, tag="msg")
            nc.tensor.matmul(msg[:], lhsT=u0s[:], rhs=sb[f"{sg}_w2a_{l}"][:],
                             start=True, stop=False)
            nc.tensor.matmul(msg[:], lhsT=u1s[:], rhs=sb[f"{sg}_w2b_{l}"][:],
                             start=False, stop=False)
            nc.tensor.matmul(msg[:], lhsT=u2s[:], rhs=sb[f"{sg}_w2c_{l}"][:],
                             start=False, stop=True)

            # groupnorm (2 groups of 64) + leaky + residual
            y = small_p.tile([P, D], F32, tag="y")
            nc.vector.tensor_tensor(out=y[:], in0=msg[:],
                                    in1=sb[f"{sg}_b2rep_{l}"][:], op=ALU.add)
            var = small_p.tile([P, 2], F32, tag="var")
            for h in range(2):
                hs = slice(h * 64, (h + 1) * 64)
                mu = small_p.tile([P, 1], F32, tag="mu")
                nc.vector.tensor_reduce(mu[:], y[:, hs], axis=AX.X, op=ALU.add)
                nc.vector.tensor_scalar_mul(mu[:], mu[:], 1.0 / 64.0)
                nc.vector.tensor_scalar(y[:, hs], y[:, hs], mu[:, 0:1], None,
                                        op0=ALU.subtract)
                sq = small_p.tile([P, 64], F32, tag="sq")
                nc.scalar.activation(sq[:], y[:, hs], ACTF.Square,
                                     accum_out=var[:, h:h + 1])
            rstd = small_p.tile([P, 2], F32, tag="rstd")
            nc.scalar.activation(rstd[:], var[:], ACTF.Sqrt, bias=eps_col[:, 0:1],
                                 scale=1.0 / 64.0)
            nc.vector.reciprocal(rstd[:], rstd[:])
            for h in range(2):
                hs = slice(h * 64, (h + 1) * 64)
                nc.vector.tensor_scalar(y[:, hs], y[:, hs], rstd[:, h:h + 1], None,
                                        op0=ALU.mult)
            nc.vector.tensor_tensor(out=y[:], in0=y[:], in1=gwrep[(sg, l)][:],
                                    op=ALU.mult)
            nc.vector.tensor_tensor(out=y[:], in0=y[:], in1=gbrep[(sg, l)][:],
                                    op=ALU.add)
            leaky_inplace(y, y[:])
            nc.vector.tensor_tensor(out=y[:], in0=y[:], in1=ctr_tiles[ct][:],
                                    op=ALU.add)
            if out_is_final:
                dma(out_ext[ct * P:(ct + 1) * P, :], y[:])
                new_tiles.append(None)
            else:
                nt = emb_p.tile([P, D], F32, tag=f"{sg}_emb{ct}_{l % 2}")
                nc.vector.tensor_copy(nt[:], y[:])
                new_tiles.append(nt)
                pst = psum2_p.tile([P, P], F32, tag="t")
                nc.tensor.transpose(pst[:], in_=nt[:], identity=identf32[:])
                nc.vector.tensor_copy(new_T[:, ct * P:(ct + 1) * P], pst[:])
        return new_T, new_tiles

    # ---- atom stage ------------------------------------------------------
    curT, cur = a_embT, a_emb
    for l in range(L):
        curT, cur = mp_layer("aa", l, curT, curT, cur, a_idx, a_dst, NTA,
                             table_dram[l % 2])
    a_finT = curT

    # ---- point stage ------------------------------------------------------
    p_embT = emb_p.tile([D, NX_S], F32, tag="p_embT")
    nc.vector.memset(p_embT[:], 1.0)
    p_emb = []
    for ct in range(NTX):
        t = emb_p.tile([P, D], F32, tag=f"p_emb{ct}")
        nc.vector.memset(t[:], 1.0)
        p_emb.append(t)
    curT, cur = p_embT, p_emb
    for l in range(L):
        curT, cur = mp_layer("ae", l, a_finT, curT, cur, x_idx, x_dst, NTX,
                             table_dram[l % 2], out_is_final=(l == L - 1))

    ctx.close()
    nc.finalize()
    return nc


# ----------------------------------------------------------------------------
# host wrapper
# ----------------------------------------------------------------------------

def kernel(**inputs):
    xyz = np.asarray(inputs["xyz"], np.float32)
    atom_xyz = np.asarray(inputs["atom_xyz"], np.float32)
    atom_features = np.asarray(inputs["atom_features"], np.float32)
    batch = np.asarray(inputs["batch"]).astype(np.int64)
    atom_batch = np.asarray(inputs["atom_batch"]).astype(np.int64)

    NX, NA, B = xyz.shape[0], atom_xyz.shape[0], 8
    cx = np.bincount(batch, minlength=B)
    ca = np.bincount(atom_batch, minlength=B)
    ox = np.concatenate([[0], np.cumsum(cx)])
    oa = np.concatenate([[0], np.cumsum(ca)])
    NA_S = max(_round_up(int(ca.max()), P), P)
    NX_S = max(_round_up(int(cx.max()), P), P)

    nc = build_graph(NA_S, NX_S)

    shared = {
        "tw1": np.ascontiguousarray(np.asarray(inputs["tw1"], np.float32)),
        "tw2": np.ascontiguousarray(np.asarray(inputs["tw2"], np.float32)),
        "tb1": np.asarray(inputs["tb1"], np.float32)[:, None].copy(),
        "tb2": np.asarray(inputs["tb2"], np.float32)[:, None].copy(),
    }
    shared.update(lay_dict(inputs, "aa"))
    shared.update(lay_dict(inputs, "ae"))
    shared["identbf"] = np.eye(P, dtype=ml_dtypes.bfloat16)
    shared["identf32"] = np.eye(P, dtype=np.float32)
    shared["epsc"] = np.full((P, 1), 1e-5, np.float32)

    in_maps = []
    for g in range(B):
        na, nx = int(ca[g]), int(cx[g])
        ax = atom_xyz[oa[g]:oa[g] + na]
        px = xyz[ox[g]:ox[g] + nx]
        af = atom_features[oa[g]:oa[g] + na]

        featT = np.zeros((D, NA_S), np.float32)
        featT[:, :na] = af.T
        a_q = np.zeros((4, NA_S), np.float32)
        a_q[3, :] = 1.0
        a_q[:3, :na] = 2.0 * ax.T
        a_db = np.zeros((4, NA_S), np.float32)
        a_db[3, :] = -1e30
        a_db[:3, :na] = ax.T
        a_db[3, :na] = -np.sum(ax * ax, axis=1)
        a_qn = np.zeros((NA_S, 1), np.float32)
        a_qn[:na, 0] = -np.sum(ax * ax, axis=1)
        x_q = np.zeros((4, NX_S), np.float32)
        x_q[3, :] = 1.0
        x_q[:3, :nx] = 2.0 * px.T
        x_qn = np.zeros((NX_S, 1), np.float32)
        x_qn[:nx, 0] = -np.sum(px * px, axis=1)

        m = {"featT": featT, "a_q": a_q, "a_db": a_db, "a_qn": a_qn,
             "x_q": x_q, "x_qn": x_qn}
        m.update(shared)
        in_maps.append(m)

    trace = bool(os.environ.get("ATOM_TRACE"))
    rr = run_bass_kernel_spmd(nc, in_maps, core_ids=list(range(B)), trace=trace)
    if trace and rr.exec_time_ns:
        print(f"HW exec time: {rr.exec_time_ns} ns", flush=True)
    if os.environ.get("ATOM_TIME"):
        import time as _t
        ts = []
        for _ in range(int(os.environ.get("ATOM_TIME", "3"))):
            t0 = _t.perf_counter()
            rr = run_bass_kernel_spmd(nc, in_maps, core_ids=list(range(B)))
            ts.append(_t.perf_counter() - t0)
        print(f"wall exec times: {[f'{t*1e3:.1f}ms' for t in ts]}", flush=True)
        print(f"HW exec time: {min(ts)*1e9:.0f} ns", flush=True)
    res = rr.results
    out = np.empty((NX, D), np.float32)
    for g in range(B):
        out[ox[g]:ox[g] + int(cx[g])] = res[g]["out"][:int(cx[g])]
    return out


def lay_dict(inputs, sg):
    w1 = np.asarray(inputs[f"{sg}_w1"], np.float32)
    b1 = np.asarray(inputs[f"{sg}_b1"], np.float32)
    w2 = np.asarray(inputs[f"{sg}_w2"], np.float32)
    b2 = np.asarray(inputs[f"{sg}_b2"], np.float32)
    gw = np.asarray(inputs[f"{sg}_gw"], np.float32)
    gb = np.asarray(inputs[f"{sg}_gb"], np.float32)
    d = {}
    for l in range(L):
        d[f"{sg}_c1w_{l}"] = np.ascontiguousarray(w1[l][:D, :])
        d[f"{sg}_s1w_{l}"] = np.ascontiguousarray(w1[l][D:2 * D, :])
        rpad = np.zeros((P, HP), np.float32)
        rpad[:, :H] = w1[l][2 * D]
        d[f"{sg}_rrep_{l}"] = rpad.astype(ml_dtypes.bfloat16)
        d[f"{sg}_b1rep_{l}"] = np.broadcast_to(b1[l], (P, H)).copy()
        d[f"{sg}_w2a_{l}"] = np.ascontiguousarray(w2[l][:D, :]).astype(ml_dtypes.bfloat16)
        d[f"{sg}_w2b_{l}"] = np.ascontiguousarray(w2[l][D:2 * D, :]).astype(ml_dtypes.bfloat16)
        d[f"{sg}_w2c_{l}"] = w2[l][2 * D:2 * D + 1, :].astype(ml_dtypes.bfloat16).copy()
        d[f"{sg}_b2rep_{l}"] = np.broadcast_to(K * b2[l], (P, D)).copy()
        d[f"{sg}_gwrep_{l}"] = np.broadcast_to(gw[l], (P, D)).copy()
        d[f"{sg}_gbrep_{l}"] = np.broadcast_to(gb[l], (P, D)).copy()
    return d


# revision 17
# speedup vs baseline: 9.1845x; 8.2169x over previous
"""AtomNet message-passing kernel for 8 Trainium2 NeuronCores.

Data-parallel over graphs: batch ids are sorted, KNN is batch-restricted, so
graph g goes to core g (8 graphs, 8 cores), no cross-core edges.

Per-core pipeline (all compute on device):
  1. transform: out = leaky(feat @ tw1 + tb1) @ tw2 + tb2        (PE + ACT/DVE)
  2. atom-atom KNN (k=16, self dropped) via PE distance matmul +
     DVE max8/max_index/match_replace rounds on negated d2
  3. 3 atom MP layers, decomposed edge MLP:
       msg_i = (sum_j leaky(C1_i + S1[idx_ij] + d_ij*r + b1)) @ w2 + 16*b2
       C1 = ctr @ w1[:128], S1 = src @ w1[128:256], r = w1[256]
     S1 table -> DRAM, per-edge rows gathered with indirect DMA (bf16),
     leaky+sum on DVE slabs, second matmul + groupnorm + residual.
  4. point-atom KNN, 3 point MP layers same way (src = final atom emb).
"""

import math
import os
from contextlib import ExitStack

import ml_dtypes
import numpy as np

import concourse.bass as bass
import concourse.bacc as bacc_mod
import concourse.mybir as mybir
import concourse.tile as tile
from concourse.bass_utils import run_bass_kernel_spmd
from concourse.masks import make_identity
from concourse.tile import TileContext

P = 128
D = 128
K = 16
L = 3
H = 2 * D + 1  # 257
HP = 264  # padded gather row (bf16 elems)
SLOPE = 0.2
NEG_BIG = -3.0e38
F32 = mybir.dt.float32
BF16 = mybir.dt.bfloat16
U32 = mybir.dt.uint32
AX = mybir.AxisListType
ALU = mybir.AluOpType
ACTF = mybir.ActivationFunctionType


def _leaky_np(z):
    return np.where(z >= 0, z, SLOPE * z)


def _round_up(n, m):
    return ((n + m - 1) // m) * m


# ----------------------------------------------------------------------------
# device graph
# ----------------------------------------------------------------------------

def build_graph(NA_S, NX_S):
    NTA = NA_S // P
    NTX = NX_S // P
    nc = bacc_mod.Bacc()

    # ---- dram parameters -------------------------------------------------
    featT = nc.declare_dram_parameter("featT", [D, NA_S], F32, isOutput=False)
    a_q = nc.declare_dram_parameter("a_q", [4, NA_S], F32, isOutput=False)
    a_db = nc.declare_dram_parameter("a_db", [4, NA_S], F32, isOutput=False)
    a_qn = nc.declare_dram_parameter("a_qn", [NA_S, 1], F32, isOutput=False)
    x_q = nc.declare_dram_parameter("x_q", [4, NX_S], F32, isOutput=False)
    x_qn = nc.declare_dram_parameter("x_qn", [NX_S, 1], F32, isOutput=False)
    tw1 = nc.declare_dram_parameter("tw1", [D, D], F32, isOutput=False)
    tw2 = nc.declare_dram_parameter("tw2", [D, D], F32, isOutput=False)
    tb1 = nc.declare_dram_parameter("tb1", [D, 1], F32, isOutput=False)
    tb2 = nc.declare_dram_parameter("tb2", [D, 1], F32, isOutput=False)
    lw = {}
    for sg in ("aa", "ae"):
        for l in range(L):
            for nm, shp in (
                ("c1w", [D, H]), ("s1w", [D, H]), ("rrep", [P, HP]),
                ("b1rep", [P, H]), ("w2a", [D, D]), ("w2b", [D, D]),
                ("w2c", [1, D]), ("b2rep", [P, D]), ("gwrep", [P, D]),
                ("gbrep", [P, D]),
            ):
                key = f"{sg}_{nm}_{l}"
                dt = BF16 if nm in ("rrep", "w2a", "w2b", "w2c") else F32
                lw[key] = nc.declare_dram_parameter(key, shp, dt, isOutput=False)
    identbf_d = nc.declare_dram_parameter("identbf", [P, P], BF16, isOutput=False)
    identf_d = nc.declare_dram_parameter("identf32", [P, P], F32, isOutput=False)
    epsc_d = nc.declare_dram_parameter("epsc", [P, 1], F32, isOutput=False)
    out_ext = nc.declare_dram_parameter("out", [NX_S, D], F32, isOutput=True)

    table_dram = [nc.dram_tensor(f"table{i}", [NA_S, HP], BF16) for i in range(2)]

    ctx = ExitStack()
    tc = ctx.enter_context(TileContext(nc))
    const_p = ctx.enter_context(tc.tile_pool(name="const", bufs=1))
    wpool = ctx.enter_context(tc.tile_pool(name="weights", bufs=1))
    emb_p = ctx.enter_context(tc.tile_pool(name="emb", bufs=1))
    big_p = ctx.enter_context(tc.tile_pool(name="big", bufs=2))
    slab_p = ctx.enter_context(tc.tile_pool(name="slab", bufs=2))
    small_p = ctx.enter_context(tc.tile_pool(name="small", bufs=3))
    knn_p = ctx.enter_context(tc.tile_pool(name="knn", bufs=1))
    psum_p = ctx.enter_context(tc.tile_pool(name="psum", bufs=2, space="PSUM"))
    psum2_p = ctx.enter_context(tc.tile_pool(name="psum2", bufs=2, space="PSUM"))
    psumc_p = ctx.enter_context(tc.tile_pool(name="psumc", bufs=2, space="PSUM"))
    psumm_p = ctx.enter_context(tc.tile_pool(name="psumm", bufs=2, space="PSUM"))

    def dma(dst, src):
        nc.sync.dma_start(out=dst, in_=src)

    # ---- constants / weights to SBUF ------------------------------------
    ident = const_p.tile([P, P], BF16, tag="identf")
    dma(ident[:], identbf_d[:])
    identf32 = const_p.tile([P, P], F32, tag="identf32")
    dma(identf32[:], identf_d[:])
    eps_col = const_p.tile([P, 1], F32, tag="eps")
    dma(eps_col[:], epsc_d[:])

    sb = {}
    for key, hnd in lw.items():
        t = wpool.tile(list(hnd.shape), hnd.dtype, tag=key)
        dma(t[:], hnd[:])
        sb[key] = t
    for nm, hnd in (("tw1", tw1), ("tw2", tw2), ("tb1", tb1), ("tb2", tb2),
                    ("featT", featT), ("a_q", a_q), ("a_db", a_db),
                    ("x_q", x_q)):
        t = wpool.tile(list(hnd.shape), F32, tag=nm)
        dma(t[:], hnd[:])
        sb[nm] = t
    # qn columns wrapped to [P, NT]
    a_qn_t = wpool.tile([P, NTA], F32, tag="a_qn")
    dma(a_qn_t[:], a_qn[:].rearrange("(t p) o -> p (t o)", p=P))
    x_qn_t = wpool.tile([P, NTX], F32, tag="x_qn")
    dma(x_qn_t[:], x_qn[:].rearrange("(t p) o -> p (t o)", p=P))

    rrep = {(sg, l): sb[f"{sg}_rrep_{l}"] for sg in ("aa", "ae") for l in range(L)}
    gwrep = {(sg, l): sb[f"{sg}_gwrep_{l}"] for sg in ("aa", "ae") for l in range(L)}
    gbrep = {(sg, l): sb[f"{sg}_gbrep_{l}"] for sg in ("aa", "ae") for l in range(L)}

    def leaky_inplace(t, ap, tmp_pool=small_p):
        """t[ap...] = leaky(t) via mul+max. ap is an AP on an SBUF tile."""
        tmp = tmp_pool.tile([ap.shape[0], ap.free_size()], ap.dtype)
        nc.vector.tensor_scalar_mul(tmp[:], ap, SLOPE)
        nc.vector.tensor_tensor(out=ap, in0=ap, in1=tmp[:], op=ALU.max)

    # ---- stage 1: transform atoms ---------------------------------------
    # a_embT [D, NA_S] f32, feature-on-partition
    a_embT = emb_p.tile([D, NA_S], F32, tag="a_embT")
    CH = 512
    for c0 in range(0, NA_S, CH):
        cw = min(CH, NA_S - c0)
        ps = psum_p.tile([P, cw], F32, tag="mm")
        nc.tensor.matmul(ps[:], lhsT=sb["tw1"][:], rhs=sb["featT"][:, c0:c0 + cw],
                         start=True, stop=True)
        h = big_p.tile([P, cw], F32, tag="h_trans")
        nc.scalar.activation(h[:], ps[:], ACTF.Identity, bias=sb["tb1"][:, 0:1])
        leaky_inplace(h, h[:], big_p)
        ps2 = psum_p.tile([P, cw], F32, tag="mm")
        nc.tensor.matmul(ps2[:], lhsT=sb["tw2"][:], rhs=h[:], start=True, stop=True)
        nc.scalar.activation(a_embT[:, c0:c0 + cw], ps2[:], ACTF.Identity,
                             bias=sb["tb2"][:, 0:1])

    # atom emb tiles [ctr, D] f32 (transpose of a_embT chunks)
    a_emb = []
    for ct in range(NTA):
        pst = psum_p.tile([P, P], F32, tag="mm")
        nc.tensor.transpose(pst[:], in_=a_embT[:, ct * P:(ct + 1) * P],
                            identity=identf32[:])
        t = emb_p.tile([P, D], F32, tag=f"a_emb{ct}")
        nc.vector.tensor_copy(t[:], pst[:])
        a_emb.append(t)

    # ---- KNN -------------------------------------------------------------
    def knn(q_aug, qn_t, NT, diag_kill):
        """returns (idx tiles [P,16] u32, dist tiles [P,16] f32) per ctr tile."""
        idx_tiles, dst_tiles = [], []
        for ct in range(NT):
            nd2 = big_p.tile([P, NA_S], F32, tag="nd2")
            for c0 in range(0, NA_S, CH):
                cw = min(CH, NA_S - c0)
                ps = psum2_p.tile([P, cw], F32, tag="t")
                nc.tensor.matmul(ps[:], lhsT=q_aug[:, ct * P:(ct + 1) * P],
                                 rhs=sb["a_db"][:, c0:c0 + cw], start=True, stop=True)
                nc.scalar.activation(nd2[:, c0:c0 + cw], ps[:], ACTF.Identity,
                                     bias=qn_t[:, ct:ct + 1])
            if diag_kill:
                nc.gpsimd.affine_select(
                    out=nd2[:], in_=nd2[:], compare_op=ALU.not_equal,
                    fill=NEG_BIG, base=-P * ct,
                    pattern=[[1, NA_S]], channel_multiplier=-1)
            idx = knn_p.tile([P, K], U32, tag=f"idx_{diag_kill}_{ct}")
            dst = knn_p.tile([P, K], F32, tag=f"dst_{diag_kill}_{ct}")
            m8 = small_p.tile([P, 8], F32, tag="m8")
            nc.vector.max(out=m8[:], in_=nd2[:])
            nc.vector.max_index(idx[:, 0:8], m8[:], nd2[:])
            nc.vector.tensor_scalar_mul(dst[:, 0:8], m8[:], -1.0)
            nc.vector.match_replace(out=nd2[:], in_to_replace=m8[:],
                                    in_values=nd2[:], imm_value=NEG_BIG)
            m8b = small_p.tile([P, 8], F32, tag="m8b")
            nc.vector.max(out=m8b[:], in_=nd2[:])
            nc.vector.max_index(idx[:, 8:16], m8b[:], nd2[:])
            nc.vector.tensor_scalar_mul(dst[:, 8:16], m8b[:], -1.0)
            idx_tiles.append(idx)
            dst_tiles.append(dst)
        return idx_tiles, dst_tiles

    a_idx, a_dst = knn(sb["a_q"], a_qn_t, NTA, diag_kill=True)
    x_idx, x_dst = knn(sb["x_q"], x_qn_t, NTX, diag_kill=False)

    # ---- MP layer --------------------------------------------------------
    def mp_layer(sg, l, srcT, ctrT, ctr_tiles, idx_tiles, dst_tiles, NT, tdram,
                 out_is_final=False):
        """One message-passing layer.
        srcT  : [D, NA_S] f32 source features (feature-on-partition)
        ctrT  : [D, NT*P] f32 center features (feature-on-partition)
        ctr_tiles: list of [P, D] f32 center tiles (residual)
        returns (new ctrT tile, new ctr tiles) unless out_is_final: writes out.
        """
        # source table -> DRAM (bf16)
        tbl = big_p.tile([P, NTA, HP], BF16, tag="tbl")
        nc.vector.memset(tbl[:, :, H:], 0.0)
        for st in range(NTA):
            ps = psum_p.tile([P, H], F32, tag="mm")
            nc.tensor.matmul(ps[:], lhsT=srcT[:, st * P:(st + 1) * P],
                             rhs=sb[f"{sg}_s1w_{l}"][:], start=True, stop=True)
            nc.vector.tensor_tensor(out=tbl[:, st, 0:H], in0=ps[:],
                                    in1=sb[f"{sg}_b1rep_{l}"][:], op=ALU.add)
        dma(tdram[:].rearrange("(t p) f -> p t f", p=P), tbl[:])

        new_T = None if out_is_final else emb_p.tile(
            [D, NT * P], F32, tag=f"{sg}_embT_{l % 2}")
        new_tiles = []
        for ct in range(NT):
            # C1 + b1 (bf16 [P, H])
            ps = psumc_p.tile([P, H], F32, tag="c1")
            nc.tensor.matmul(ps[:], lhsT=ctrT[:, ct * P:(ct + 1) * P],
                             rhs=sb[f"{sg}_c1w_{l}"][:], start=True, stop=True)
            c1 = small_p.tile([P, H], BF16, tag="c1")
            nc.scalar.copy(c1[:], ps[:])

            # base slab: g[:, j, :] = d_j * r (rrep padded, zeros past H), += c1
            g = slab_p.tile([P, K, HP], BF16, tag="g")
            for j in range(K):
                nc.vector.tensor_scalar_mul(g[:, j, :], rrep[(sg, l)][:],
                                            dst_tiles[ct][:, j:j + 1])
            for j in range(K):
                nc.vector.tensor_tensor(out=g[:, j, 0:H], in0=g[:, j, 0:H],
                                        in1=c1[:], op=ALU.add)
            # gather-accumulate source rows on top: z = base + s1[idx]
            for j in range(K):
                nc.gpsimd.indirect_dma_start(
                    out=g[:, j, :], out_offset=None, in_=tdram[:],
                    in_offset=bass.IndirectOffsetOnAxis(
                        ap=idx_tiles[ct][:, j:j + 1], axis=0),
                    compute_op=ALU.add)
            # leaky: max(z, 0.2 z); 0.2*z computed on ACT
            lk = slab_p.tile([P, K, HP], BF16, tag="lk")
            nc.scalar.activation(lk[:], g[:], ACTF.Identity, scale=0.2)
            nc.vector.tensor_tensor(out=g[:], in0=g[:], in1=lk[:], op=ALU.max)
            for step in (8, 4, 2, 1):
                nc.vector.tensor_tensor(out=g[:, 0:step, :], in0=g[:, 0:step, :],
                                        in1=g[:, step:2 * step, :], op=ALU.add)
            # u = g[:, 0, 0:H] bf16 -> transpose chunks
            u0 = psum2_p.tile([P, P], BF16, tag="t")
            nc.tensor.transpose(u0[:], in_=g[:, 0, 0:P], identity=ident[:])
            u1 = psum2_p.tile([P, P], BF16, tag="t")
            nc.tensor.transpose(u1[:], in_=g[:, 0, P:2 * P], identity=ident[:])
            u2 = psum2_p.tile([1, P], BF16, tag="t")
            nc.tensor.transpose(u2[:], in_=g[:, 0, 2 * P:H], identity=ident[:])
            u0s = small_p.tile([P, P], BF16, tag="u0s")
            nc.vector.tensor_copy(u0s[:], u0[:])
            u1s = small_p.tile([P, P], BF16, tag="u1s")
            nc.vector.tensor_copy(u1s[:], u1[:])
            u2s = small_p.tile([1, P], BF16, tag="u2s")
            nc.vector.tensor_copy(u2s[:], u2[:])

            msg = psumm_p.tile([P, D], F32, tag="msg")
            nc.tensor.matmul(msg[:], lhsT=u0s[:], rhs=sb[f"{sg}_w2a_{l}"][:],
                             start=True, stop=False)
            nc.tensor.matmul(msg[:], lhsT=u1s[:], rhs=sb[f"{sg}_w2b_{l}"][:],
                             start=False, stop=False)
            nc.tensor.matmul(msg[:], lhsT=u2s[:], rhs=sb[f"{sg}_w2c_{l}"][:],
                             start=False, stop=True)

            # groupnorm (2 groups of 64) + leaky + residual
            y = small_p.tile([P, D], F32, tag="y")
            nc.vector.tensor_tensor(out=y[:], in0=msg[:],
                                    in1=sb[f"{sg}_b2rep_{l}"][:], op=ALU.add)
            var = small_p.tile([P, 2], F32, tag="var")
            for h in range(2):
                hs = slice(h * 64, (h + 1) * 64)
                mu = small_p.tile([P, 1], F32, tag="mu")
                nc.vector.tensor_reduce(mu[:], y[:, hs], axis=AX.X, op=ALU.add)
                nc.vector.tensor_scalar_mul(mu[:], mu[:], 1.0 / 64.0)
                nc.vector.tensor_scalar(y[:, hs], y[:, hs], mu[:, 0:1], None,
                                        op0=ALU.subtract)
                sq = small_p.tile([P, 64], F32, tag="sq")
                nc.scalar.activation(sq[:], y[:, hs], ACTF.Square,
                                     accum_out=var[:, h:h + 1])
            rstd = small_p.tile([P, 2], F32, tag="rstd")
            nc.scalar.activation(rstd[:], var[:], ACTF.Sqrt, bias=eps_col[:, 0:1],
                                 scale=1.0 / 64.0)
            nc.vector.reciprocal(rstd[:], rstd[:])
            for h in range(2):
                hs = slice(h * 64, (h + 1) * 64)
                nc.vector.tensor_scalar(y[:, hs], y[:, hs], rstd[:, h:h + 1], None,
                                        op0=ALU.mult)
            nc.vector.tensor_tensor(out=y[:], in0=y[:], in1=gwrep[(sg, l)][:],
                                    op=ALU.mult)
            nc.vector.tensor_tensor(out=y[:], in0=y[:], in1=gbrep[(sg, l)][:],
                                    op=ALU.add)
            leaky_inplace(y, y[:])
            nc.vector.tensor_tensor(out=y[:], in0=y[:], in1=ctr_tiles[ct][:],
                                    op=ALU.add)
            if out_is_final:
                dma(out_ext[ct * P:(ct + 1) * P, :], y[:])
                new_tiles.append(None)
            else:
                nt = emb_p.tile([P, D], F32, tag=f"{sg}_emb{ct}_{l % 2}")
                nc.vector.tensor_copy(nt[:], y[:])
                new_tiles.append(nt)
                pst = psum2_p.tile([P, P], F32, tag="t")
                nc.tensor.transpose(pst[:], in_=nt[:], identity=identf32[:])
                nc.vector.tensor_copy(new_T[:, ct * P:(ct + 1) * P], pst[:])
        return new_T, new_tiles

    # ---- atom stage ------------------------------------------------------
    curT, cur = a_embT, a_emb
    for l in range(L):
        curT, cur = mp_layer("aa", l, curT, curT, cur, a_idx, a_dst, NTA,
                             table_dram[l % 2])
    a_finT = curT

    # ---- point stage ------------------------------------------------------
    p_embT = emb_p.tile([D, NX_S], F32, tag="p_embT")
    nc.vector.memset(p_embT[:], 1.0)
    p_emb = []
    for ct in range(NTX):
        t = emb_p.tile([P, D], F32, tag=f"p_emb{ct}")
        nc.vector.memset(t[:], 1.0)
        p_emb.append(t)
    curT, cur = p_embT, p_emb
    for l in range(L):
        curT, cur = mp_layer("ae", l, a_finT, curT, cur, x_idx, x_dst, NTX,
                             table_dram[l % 2], out_is_final=(l == L - 1))

    ctx.close()
    nc.finalize()
    return nc


def build_null_graph(NA_S, NX_S):
    """Same I/O signature as build_graph, trivial compute: calibrates overhead."""
    NTX = NX_S // P
    nc = bacc_mod.Bacc()
    names = [("featT", [D, NA_S], F32), ("a_q", [4, NA_S], F32),
             ("a_db", [4, NA_S], F32), ("a_qn", [NA_S, 1], F32),
             ("x_q", [4, NX_S], F32), ("x_qn", [NX_S, 1], F32),
             ("tw1", [D, D], F32), ("tw2", [D, D], F32),
             ("tb1", [D, 1], F32), ("tb2", [D, 1], F32),
             ("identbf", [P, P], BF16), ("identf32", [P, P], F32),
             ("epsc", [P, 1], F32)]
    for sg in ("aa", "ae"):
        for l in range(L):
            for nm, shp in (("c1w", [D, H]), ("s1w", [D, H]), ("rrep", [P, HP]),
                            ("b1rep", [P, H]), ("w2a", [D, D]), ("w2b", [D, D]),
                            ("w2c", [1, D]), ("b2rep", [P, D]), ("gwrep", [P, D]),
                            ("gbrep", [P, D])):
                dt = BF16 if nm in ("rrep", "w2a", "w2b", "w2c") else F32
                names.append((f"{sg}_{nm}_{l}", shp, dt))
    hs = {}
    for nm, shp, dt in names:
        hs[nm] = nc.declare_dram_parameter(nm, shp, dt, isOutput=False)
    out_ext = nc.declare_dram_parameter("out", [NX_S, D], F32, isOutput=True)
    ctx = ExitStack()
    tc = ctx.enter_context(TileContext(nc))
    pool = ctx.enter_context(tc.tile_pool(name="p", bufs=1))
    t = pool.tile([P, D], F32)
    nc.sync.dma_start(out=t[:], in_=hs["featT"][:, 0:D])
    for i in range(NTX):
        nc.sync.dma_start(out=out_ext[i * P:(i + 1) * P, :], in_=t[:])
    ctx.close()
    nc.finalize()
    return nc


# ----------------------------------------------------------------------------
# host wrapper
# ----------------------------------------------------------------------------

def kernel(**inputs):
    xyz = np.asarray(inputs["xyz"], np.float32)
    atom_xyz = np.asarray(inputs["atom_xyz"], np.float32)
    atom_features = np.asarray(inputs["atom_features"], np.float32)
    batch = np.asarray(inputs["batch"]).astype(np.int64)
    atom_batch = np.asarray(inputs["atom_batch"]).astype(np.int64)

    NX, NA, B = xyz.shape[0], atom_xyz.shape[0], 8
    cx = np.bincount(batch, minlength=B)
    ca = np.bincount(atom_batch, minlength=B)
    ox = np.concatenate([[0], np.cumsum(cx)])
    oa = np.concatenate([[0], np.cumsum(ca)])
    NA_S = max(_round_up(int(ca.max()), P), P)
    NX_S = max(_round_up(int(cx.max()), P), P)

    nc = build_graph(NA_S, NX_S)

    shared = {
        "tw1": np.ascontiguousarray(np.asarray(inputs["tw1"], np.float32)),
        "tw2": np.ascontiguousarray(np.asarray(inputs["tw2"], np.float32)),
        "tb1": np.asarray(inputs["tb1"], np.float32)[:, None].copy(),
        "tb2": np.asarray(inputs["tb2"], np.float32)[:, None].copy(),
    }
    shared.update(lay_dict(inputs, "aa"))
    shared.update(lay_dict(inputs, "ae"))
    shared["identbf"] = np.eye(P, dtype=ml_dtypes.bfloat16)
    shared["identf32"] = np.eye(P, dtype=np.float32)
    shared["epsc"] = np.full((P, 1), 1e-5, np.float32)

    in_maps = []
    for g in range(B):
        na, nx = int(ca[g]), int(cx[g])
        ax = atom_xyz[oa[g]:oa[g] + na]
        px = xyz[ox[g]:ox[g] + nx]
        af = atom_features[oa[g]:oa[g] + na]

        featT = np.zeros((D, NA_S), np.float32)
        featT[:, :na] = af.T
        a_q = np.zeros((4, NA_S), np.float32)
        a_q[3, :] = 1.0
        a_q[:3, :na] = 2.0 * ax.T
        a_db = np.zeros((4, NA_S), np.float32)
        a_db[3, :] = -1e30
        a_db[:3, :na] = ax.T
        a_db[3, :na] = -np.sum(ax * ax, axis=1)
        a_qn = np.zeros((NA_S, 1), np.float32)
        a_qn[:na, 0] = -np.sum(ax * ax, axis=1)
        x_q = np.zeros((4, NX_S), np.float32)
        x_q[3, :] = 1.0
        x_q[:3, :nx] = 2.0 * px.T
        x_qn = np.zeros((NX_S, 1), np.float32)
        x_qn[:nx, 0] = -np.sum(px * px, axis=1)

        m = {"featT": featT, "a_q": a_q, "a_db": a_db, "a_qn": a_qn,
             "x_q": x_q, "x_qn": x_qn}
        m.update(shared)
        in_maps.append(m)

    trace = bool(os.environ.get("ATOM_TRACE"))
    rr = run_bass_kernel_spmd(nc, in_maps, core_ids=list(range(B)), trace=trace)
    if trace and rr.exec_time_ns:
        print(f"HW exec time: {rr.exec_time_ns} ns", flush=True)
    if os.environ.get("ATOM_TIME"):
        import time as _t
        n_rep = int(os.environ.get("ATOM_TIME", "3"))
        ts = []
        for _ in range(n_rep):
            t0 = _t.perf_counter()
            rr = run_bass_kernel_spmd(nc, in_maps, core_ids=list(range(B)))
            ts.append(_t.perf_counter() - t0)
        print(f"wall exec times: {[f'{t*1e3:.1f}ms' for t in ts]}", flush=True)
        # null graph with identical params/outputs to subtract transfer+dispatch
        nc0 = build_null_graph(NA_S, NX_S)
        t0s = []
        for _ in range(n_rep):
            t0 = _t.perf_counter()
            run_bass_kernel_spmd(nc0, in_maps, core_ids=list(range(B)))
            t0s.append(_t.perf_counter() - t0)
        print(f"null wall times: {[f'{t*1e3:.1f}ms' for t in t0s]}", flush=True)
        est = min(ts) - min(t0s)
        print(f"HW exec time: {est*1e9:.0f} ns (wall-difference estimate)", flush=True)
    res = rr.results
    out = np.empty((NX, D), np.float32)
    for g in range(B):
        out[ox[g]:ox[g] + int(cx[g])] = res[g]["out"][:int(cx[g])]
    return out


def lay_dict(inputs, sg):
    w1 = np.asarray(inputs[f"{sg}_w1"], np.float32)
    b1 = np.asarray(inputs[f"{sg}_b1"], np.float32)
    w2 = np.asarray(inputs[f"{sg}_w2"], np.float32)
    b2 = np.asarray(inputs[f"{sg}_b2"], np.float32)
    gw = np.asarray(inputs[f"{sg}_gw"], np.float32)
    gb = np.asarray(inputs[f"{sg}_gb"], np.float32)
    d = {}
    for l in range(L):
        d[f"{sg}_c1w_{l}"] = np.ascontiguousarray(w1[l][:D, :])
        d[f"{sg}_s1w_{l}"] = np.ascontiguousarray(w1[l][D:2 * D, :])
        rpad = np.zeros((P, HP), np.float32)
        rpad[:, :H] = w1[l][2 * D]
        d[f"{sg}_rrep_{l}"] = rpad.astype(ml_dtypes.bfloat16)
        d[f"{sg}_b1rep_{l}"] = np.broadcast_to(b1[l], (P, H)).copy()
        d[f"{sg}_w2a_{l}"] = np.ascontiguousarray(w2[l][:D, :]).astype(ml_dtypes.bfloat16)
        d[f"{sg}_w2b_{l}"] = np.ascontiguousarray(w2[l][D:2 * D, :]).astype(ml_dtypes.bfloat16)
        d[f"{sg}_w2c_{l}"] = w2[l][2 * D:2 * D + 1, :].astype(ml_dtypes.bfloat16).copy()
        d[f"{sg}_b2rep_{l}"] = np.broadcast_to(K * b2[l], (P, D)).copy()
        d[f"{sg}_gwrep_{l}"] = np.broadcast_to(gw[l], (P, D)).copy()
        d[f"{sg}_gbrep_{l}"] = np.broadcast_to(gb[l], (P, D)).copy()
    return d
